# revision 16
# baseline (speedup 1.0000x reference)
"""Trainium2 Bass kernel for nn_Attention_3934190044008.

Multi-head attention with additive bias and sigmoid gating:
  q = (q_x @ w_q) / 8, k = kv_x @ w_k, v = kv_x @ w_v   (8 heads x 64)
  a = softmax(q k^T + bias);  o = a @ v
  o = o * sigmoid(q_x @ w_g + b_g);  out = o @ w_o + b_o

Sharding: 16 (batch, head) pairs over 8 cores -> each core owns one batch
element and 2 heads, produces per-head transposed partials
outT_h = w_o_h^T (g_h * o_h_unnorm) [256, 2048] plus exp-sum rows; the host
divides by the exp sums, transposes, sums the 4 head-partials per batch and
adds b_o.

All matmuls run in bf16 (measured 0.75 ns/col on this silicon vs 1.15 for
f32r and 2.07 for fp32): scores are a single bf16 pass accumulated in f32
PSUM (end-to-end rel err ~7e-3 vs the 2e-2 gate). The additive bias is
DVE-added in place in PSUM, exp'd on ACT straight to bf16 E tiles, and the
softmax denominator rides the AV matmul via a ones-column in V. Scores are
S^T [k, q] so softmax-over-k never needs a partition-axis reduction.

HAM discipline (the PE clock halves after any >3.4us idle gap): a dummy
matmul burst warms the clock while the input DMAs land, K/Q projections roll
straight into the attention blocks, and the V/G projections + head-0 output
projections are emitted as PE filler inside the attention loop instead of as
standalone phases. Per (h, kt) block the PE does 4 S matmuls then the
DEFERRED AV block of kt-1 (E tiles ready a full block ago -> no exp-latency
stalls, 2 LDWEIGHTS per block).

DMA: all queues share ~360 GB/s, so the two bias streams (sync HWDGE even
k-tiles / gpsimd SWDGE odd k-tiles, ~300 GB/s each when solo) are emitted
only after the startup loads on their queue; the 14-buffer SBUF ring
backpressures them. The scalar HWDGE queue only carries the three batched
output stores so its engine pipeline stays free for the exp stream.
"""

import os
import sys
import threading
from contextlib import ExitStack

import numpy as np
import ml_dtypes

_REPO = "/opt/trn_rl_repo"
if _REPO not in sys.path and os.path.isdir(_REPO):
    sys.path.insert(0, _REPO)

import concourse.bass as bass  # noqa: E402
import concourse.mybir as mybir  # noqa: E402
import concourse.tile as tile  # noqa: E402
from concourse import bacc  # noqa: E402
from concourse.bass_utils import run_bass_kernel_spmd  # noqa: E402

F32 = mybir.dt.float32
F32R = mybir.dt.float32r
BF16 = mybir.dt.bfloat16
BF16_NP = ml_dtypes.bfloat16

B, SEQ, CQ = 2, 2048, 256
H, DH = 8, 64
HD = H * DH  # 512
N_CORES = 8
HPC = 2  # heads per core
P = 128
NKT = SEQ // P  # 16 k-tiles
QB = 512
NQB = SEQ // QB  # 4 q-blocks per k-tile row


def build_nc():
    nc = bacc.Bacc("TRN2", target_bir_lowering=False, debug=False)

    qxT = nc.dram_tensor("qxT", [CQ, SEQ], BF16, kind="ExternalInput").ap()
    kvxT = nc.dram_tensor("kvxT", [CQ, SEQ], BF16, kind="ExternalInput").ap()
    biasT = nc.dram_tensor("biasT", [HPC, SEQ, SEQ], F32, kind="ExternalInput").ap()
    wq = nc.dram_tensor("wq", [CQ, HPC * DH], BF16, kind="ExternalInput").ap()
    wk = nc.dram_tensor("wk", [CQ, HPC * DH], BF16, kind="ExternalInput").ap()
    wv = nc.dram_tensor("wv", [CQ, HPC * DH], BF16, kind="ExternalInput").ap()
    wg = nc.dram_tensor("wg", [CQ, HPC * DH], BF16, kind="ExternalInput").ap()
    bg = nc.dram_tensor("bg", [HPC * DH, 1], F32, kind="ExternalInput").ap()
    wo = nc.dram_tensor("wo", [DH, HPC, 2, P], BF16, kind="ExternalInput").ap()
    # per-head unnormalized transposed partials + softmax denominators; the
    # division and cross-core summation happen on the host after the gather
    outs_d = [nc.dram_tensor(f"outT{h}", [CQ, SEQ], BF16, kind="ExternalOutput").ap()
              for h in range(HPC)]
    rs_d = nc.dram_tensor("rs", [1, HPC, SEQ], F32, kind="ExternalOutput").ap()

    EXP = mybir.ActivationFunctionType.Exp
    SIG = mybir.ActivationFunctionType.Sigmoid

    with tile.TileContext(nc) as tc:
        with ExitStack() as ctx:
            singles = ctx.enter_context(tc.tile_pool(name="singles", bufs=1))

            # ---- startup loads: K-path deps on sync, Q-path + the rest on
            # gpsimd, so both projection chains unblock in ~6us and neither
            # queue's bias stream starts before its loads are done. The
            # scalar queue stays empty (all queues share HBM bandwidth; its
            # engine pipeline must stay free for the exp stream).
            w_sbs = {}
            for name, src, eng in (("wk", wk, nc.sync), ("wq", wq, nc.gpsimd)):
                t = singles.tile([P, 2, P], BF16, tag=f"w_{name}")
                eng.dma_start(t, src.rearrange("(a p) c -> p a c", p=P))
                w_sbs[name] = t
            kvxT_sb = singles.tile([P, 2, SEQ], BF16)
            qxT_sb = singles.tile([P, 2, SEQ], BF16)
            for half in range(2):
                cs = bass.ts(half, SEQ // 2)
                nc.sync.dma_start(
                    kvxT_sb[:, :, cs],
                    kvxT.rearrange("(a p) c -> p a c", p=P)[:, :, cs])
                nc.gpsimd.dma_start(
                    qxT_sb[:, :, cs],
                    qxT.rearrange("(a p) c -> p a c", p=P)[:, :, cs])
            for name, src in (("wv", wv), ("wg", wg)):
                t = singles.tile([P, 2, P], BF16, tag=f"w_{name}")
                nc.gpsimd.dma_start(t, src.rearrange("(a p) c -> p a c", p=P))
                w_sbs[name] = t
            bg_sb = singles.tile([P, 1], F32)
            nc.gpsimd.dma_start(bg_sb, bg)
            wo_sb = singles.tile([DH, HPC, 2, P], BF16)
            nc.gpsimd.dma_start(wo_sb, wo)

            KT = singles.tile([P, SEQ], BF16)   # [2h x 64 d, k]
            QT = singles.tile([P, SEQ], BF16)   # [2h x 64 d, q]
            GT = singles.tile([P, SEQ], F32)    # gate, [2h x 64, q]
            V_sb = singles.tile([P, HPC, NKT, DH + 1], F32R)  # [k%128, h, kt, d|1]
            OG_sb = singles.tile([DH, HPC, SEQ], BF16)  # (o * g)^T per head
            rs_sb = singles.tile([1, HPC, SEQ], F32)    # softmax denominators
            os_sb = [singles.tile([P, 2, SEQ], BF16, tag=f"os{h}",
                                  name=f"os{h}")
                     for h in range(HPC)]  # outT staging [c%128, half, q]
            dummy = singles.tile([P, QB], BF16)
            probe = singles.tile([P, 2], BF16)
            nc.vector.memset(V_sb[:, :, :, DH:DH + 1].bitcast(F32), 1.0)
            nc.vector.memset(dummy, 1.0)

            biaspool = ctx.enter_context(tc.tile_pool(name="biasp", bufs=12))
            epool = ctx.enter_context(tc.tile_pool(name="ep", bufs=6))
            sbpool = ctx.enter_context(tc.tile_pool(name="sbp", bufs=4))
            spool = ctx.enter_context(
                tc.tile_pool(name="spsum", bufs=4, space="PSUM"))
            otpool = ctx.enter_context(
                tc.tile_pool(name="otpsum", bufs=2, space="PSUM"))

            def bias_dma(h, kt):
                t = biaspool.tile([P, SEQ], F32, tag="bias")
                if h == 0 and kt < 4:
                    # ring head start on the otherwise-idle scalar queue
                    # (dispatched before any exp reaches the scalar engine)
                    eng = nc.scalar
                else:
                    eng = nc.sync if kt % 2 == 0 else nc.gpsimd
                eng.dma_start(t, biasT[h, bass.ts(kt, P), :])
                return t

            # gate the scalar queue's bias head start behind the input loads
            # (all queues share HBM bandwidth: an eager scalar stream starves
            # the critical kvxT/qxT transfers and stalls the projections)
            nc.scalar.copy(probe[:, 0:1], kvxT_sb[:, 0, 0:1])
            nc.scalar.copy(probe[:, 1:2], qxT_sb[:, 0, 0:1])

            # head 0's bias tiles stream right behind the startup loads;
            # head 1's are emitted inside head 0's loop (ring-depth lookahead)
            bias_t = [[None] * NKT for _ in range(HPC)]
            for kt in range(NKT):
                bias_t[0][kt] = bias_dma(0, kt)

            # ---- HAM warm-up: dummy matmuls while the input DMAs land, so
            # the PE clock is at 2.4 GHz when the projections start and
            # never sees a >1.7us gap again (the warm-state idle window).
            def emit_dummy(n):
                for _ in range(n):
                    ps = spool.tile([P, QB], F32, tag="s", name="warm")
                    nc.tensor.matmul(ps, dummy[:, 0:P], dummy,
                                     start=True, stop=True)

            emit_dummy(14)

            # ---- K/Q/G projections (V is emitted later as PE filler inside
            # head 0's attention loop). G runs here so ALL its sigmoids hit
            # the ACT engine before the first exp: a sigmoid interleaved
            # between exps forces a ~1.3us activation-table reload per switch.
            for wt, x_sb, dst in ((w_sbs["wk"], kvxT_sb, KT),
                                  (w_sbs["wq"], qxT_sb, QT)):
                for tt in range(SEQ // QB):
                    ps = spool.tile([P, QB], F32, tag="s")
                    nc.tensor.matmul(ps, wt[:, 0, :], x_sb[:, 0, bass.ts(tt, QB)],
                                     start=True, stop=False)
                    nc.tensor.matmul(ps, wt[:, 1, :], x_sb[:, 1, bass.ts(tt, QB)],
                                     start=False, stop=True)
                    nc.vector.tensor_copy(dst[:, bass.ts(tt, QB)], ps)

            def emit_vproj(g):
                # V out rows = tokens(k), cols = [kt-group x 2 heads x 64]
                ps = spool.tile([P, QB], F32, tag="s")
                for j in range(4):
                    kt = g * 4 + j
                    nc.tensor.matmul(ps[:, bass.ts(j, P)],
                                     kvxT_sb[:, 0, bass.ts(kt, P)], w_sbs["wv"][:, 0, :],
                                     start=True, stop=False)
                    nc.tensor.matmul(ps[:, bass.ts(j, P)],
                                     kvxT_sb[:, 1, bass.ts(kt, P)], w_sbs["wv"][:, 1, :],
                                     start=False, stop=True)
                nc.vector.tensor_copy(
                    V_sb[:, :, bass.ds(g * 4, 4), 0:DH],
                    ps.rearrange("p (a h d) -> p h a d", a=4, h=HPC))

            for tt in range(SEQ // QB):
                ps = spool.tile([P, QB], F32, tag="s", name="gproj_ps")
                nc.tensor.matmul(ps, w_sbs["wg"][:, 0, :],
                                 qxT_sb[:, 0, bass.ts(tt, QB)],
                                 start=True, stop=False)
                nc.tensor.matmul(ps, w_sbs["wg"][:, 1, :],
                                 qxT_sb[:, 1, bass.ts(tt, QB)],
                                 start=False, stop=True)
                nc.scalar.activation(GT[:, bass.ts(tt, QB)], ps, SIG, bias=bg_sb)

            def emit_av(h, kt, Es, OTs):
                for qb in range(NQB):
                    nc.tensor.matmul(
                        OTs[qb // 2][:, bass.ts(qb % 2, QB)],
                        V_sb[:, h, kt, :], Es[qb],
                        start=(kt == 0), stop=(kt == NKT - 1))

            def emit_outproj(h, half, qb, cast_eng):
                ps = spool.tile([P, QB], F32, tag="s", name="fin_ps")
                nc.tensor.matmul(ps, wo_sb[:, h, half, :],
                                 OG_sb[:, h, bass.ts(qb, QB)],
                                 start=True, stop=True)
                cast_eng_fn = (nc.scalar.copy if cast_eng == "act"
                               else lambda o, i: nc.vector.tensor_copy(o, i))
                cast_eng_fn(os_sb[h][:, half, bass.ts(qb, QB)], ps)

            hsl = [slice(h * DH, (h + 1) * DH) for h in range(HPC)]
            for h in range(HPC):
                OTs = [otpool.tile([DH + 1, 2 * QB], F32, name=f"OT{h}_{i}",
                                   tag="ot") for i in range(2)]
                prevE = None
                for kt in range(NKT):
                    # PE filler ahead of each S block: V projections during
                    # head 0, head 0's output projections during head 1
                    if h == 0:
                        if kt < 4:
                            emit_vproj(kt)
                    else:
                        if kt == 0:
                            emit_outproj(0, 0, 0, "act")
                            emit_outproj(0, 0, 1, "act")
                        elif kt in (2, 4, 6, 8, 10, 12):
                            i = kt // 2 + 1
                            emit_outproj(0, i // 4, i % 4, "act")
                    Ss = []
                    for qb in range(NQB):
                        S = spool.tile([P, QB], F32, tag="s")
                        nc.tensor.matmul(S, KT[hsl[h], bass.ts(kt, P)],
                                         QT[hsl[h], bass.ts(qb, QB)],
                                         start=True, stop=True)
                        Ss.append(S)
                    if prevE is not None:
                        emit_av(h, kt - 1, prevE, OTs)
                    Es = []
                    for qb in range(NQB):
                        SB = sbpool.tile([P, QB], F32, tag="sb")
                        nc.vector.tensor_add(SB, Ss[qb],
                                             bias_t[h][kt][:, bass.ts(qb, QB)])
                        E = epool.tile([P, QB], F32R, tag="e")
                        nc.scalar.activation(E, SB, EXP)
                        Es.append(E)
                    prevE = Es
                    if h == 0:
                        bias_t[1][kt] = bias_dma(1, kt)
                emit_av(h, NKT - 1, prevE, OTs)
                # keepalive: pad the exp-chain/drain latency so the warm-state
                # HAM never sees a >1.7us PE gap at the head transition
                emit_dummy(5)
                # drain: exp-sum rows on ACT, gate muls on DVE; head 1's
                # output projections interleave with its drain halves
                for i in range(2):
                    qsl = bass.ts(i, 2 * QB)
                    nc.scalar.copy(rs_sb[:, h, qsl], OTs[i][DH:DH + 1, :])
                    nc.vector.tensor_mul(OG_sb[:, h, qsl], GT[hsl[h], qsl],
                                         OTs[i][0:DH, :])
                    if h == 1:
                        emit_dummy(3)
                        for half in range(2):
                            for qb in (2 * i, 2 * i + 1):
                                emit_outproj(1, half, qb,
                                             "act" if qb % 2 == 0 else "dve")

            # tail: the batched output stores
            nc.scalar.dma_start(
                outs_d[0].rearrange("(a p) c -> p a c", p=P), os_sb[0])
            nc.scalar.dma_start(outs_d[1][0:P, :], os_sb[1][:, 0, :])
            nc.sync.dma_start(outs_d[1][P:CQ, :], os_sb[1][:, 1, :])
            nc.sync.dma_start(rs_d, rs_sb)

    nc.compile()
    return nc


_NC = None
_NC_LOCK = threading.Lock()


def _get_nc():
    global _NC
    with _NC_LOCK:
        if _NC is None:
            _NC = build_nc()
        return _NC


def make_in_maps(q_x, kv_x, bias, w_q, w_k, w_v, w_g, b_g, w_o, b_o):
    del b_o  # added on the host after the gather
    q_x = np.asarray(q_x, dtype=np.float32)
    kv_x = np.asarray(kv_x, dtype=np.float32)
    bias = np.asarray(bias, dtype=np.float32)
    w_q = np.asarray(w_q, dtype=np.float32) * np.float32(0.125)  # fold 1/sqrt(64)
    w_k = np.asarray(w_k, dtype=np.float32)
    w_v = np.asarray(w_v, dtype=np.float32)
    w_g = np.asarray(w_g, dtype=np.float32)
    b_g = np.asarray(b_g, dtype=np.float32)
    w_o = np.asarray(w_o, dtype=np.float32)

    in_maps = []
    for c in range(N_CORES):
        b = c // (N_CORES // B)
        h0 = HPC * (c % (N_CORES // B))
        cols = slice(h0 * DH, (h0 + HPC) * DH)
        in_maps.append({
            "qxT": np.ascontiguousarray(q_x[b].T).astype(BF16_NP),
            "kvxT": np.ascontiguousarray(kv_x[b].T).astype(BF16_NP),
            "biasT": np.ascontiguousarray(bias[b, h0:h0 + HPC].swapaxes(1, 2)),
            "wq": np.ascontiguousarray(w_q[:, cols]).astype(BF16_NP),
            "wk": np.ascontiguousarray(w_k[:, cols]).astype(BF16_NP),
            "wv": np.ascontiguousarray(w_v[:, cols]).astype(BF16_NP),
            "wg": np.ascontiguousarray(w_g[:, cols]).astype(BF16_NP),
            "bg": np.ascontiguousarray(b_g[cols].reshape(HPC * DH, 1)),
            "wo": np.ascontiguousarray(
                w_o[cols, :].reshape(HPC, DH, 2, P).transpose(1, 0, 2, 3)
            ).astype(BF16_NP),
        })
    return in_maps


def gather_output(results, b_o):
    full = np.zeros((B, SEQ, CQ), dtype=np.float32)
    for c in range(N_CORES):
        b = c // (N_CORES // B)
        rs = results[c]["rs"][0]
        for h in range(HPC):
            outT = results[c][f"outT{h}"].astype(np.float32)  # [256, 2048]
            full[b] += (outT / rs[h][None, :]).T
    full += np.asarray(b_o, dtype=np.float32)
    return full


def kernel(**inputs):
    nc = _get_nc()
    in_maps = make_in_maps(**inputs)
    res = run_bass_kernel_spmd(nc, in_maps, core_ids=list(range(N_CORES)))
    return gather_output(res.results, inputs["b_o"])


# revision 17
# speedup vs baseline: 1.1654x; 1.1654x over previous
"""Trainium2 Bass kernel for nn_Attention_3934190044008.

Multi-head attention with additive bias and sigmoid gating:
  q = (q_x @ w_q) / 8, k = kv_x @ w_k, v = kv_x @ w_v   (8 heads x 64)
  a = softmax(q k^T + bias);  o = a @ v
  o = o * sigmoid(q_x @ w_g + b_g);  out = o @ w_o + b_o

Sharding: 16 (batch, head) pairs over 8 cores -> each core owns one batch
element and 2 heads, produces per-head transposed partials
outT_h = w_o_h^T (g_h * o_h_unnorm) [256, 2048] plus exp-sum rows; the host
divides by the exp sums, transposes, sums the 4 head-partials per batch and
adds b_o.

All matmuls run in bf16 (measured 0.75 ns/col on this silicon vs 1.15 for
f32r and 2.07 for fp32): scores are a single bf16 pass accumulated in f32
PSUM (end-to-end rel err ~7e-3 vs the 2e-2 gate). The additive bias is
DVE-added in place in PSUM, exp'd on ACT straight to bf16 E tiles, and the
softmax denominator rides the AV matmul via a ones-column in V. Scores are
S^T [k, q] so softmax-over-k never needs a partition-axis reduction.

HAM discipline (the PE clock halves after any >3.4us idle gap): a dummy
matmul burst warms the clock while the input DMAs land, K/Q projections roll
straight into the attention blocks, and the V/G projections + head-0 output
projections are emitted as PE filler inside the attention loop instead of as
standalone phases. Per (h, kt) block the PE does 4 S matmuls then the
DEFERRED AV block of kt-1 (E tiles ready a full block ago -> no exp-latency
stalls, 2 LDWEIGHTS per block).

DMA: all queues share ~360 GB/s, so the two bias streams (sync HWDGE even
k-tiles / gpsimd SWDGE odd k-tiles, ~300 GB/s each when solo) are emitted
only after the startup loads on their queue; the 14-buffer SBUF ring
backpressures them. The scalar HWDGE queue only carries the three batched
output stores so its engine pipeline stays free for the exp stream.
"""

import os
import sys
import threading
from contextlib import ExitStack

import numpy as np
import ml_dtypes

_REPO = "/opt/trn_rl_repo"
if _REPO not in sys.path and os.path.isdir(_REPO):
    sys.path.insert(0, _REPO)

import concourse.bass as bass  # noqa: E402
import concourse.mybir as mybir  # noqa: E402
import concourse.tile as tile  # noqa: E402
from concourse import bacc  # noqa: E402
from concourse.bass_utils import run_bass_kernel_spmd  # noqa: E402

F32 = mybir.dt.float32
F32R = mybir.dt.float32r
BF16 = mybir.dt.bfloat16
BF16_NP = ml_dtypes.bfloat16

B, SEQ, CQ = 2, 2048, 256
H, DH = 8, 64
HD = H * DH  # 512
N_CORES = 8
HPC = 2  # heads per core
P = 128
NKT = SEQ // P  # 16 k-tiles
QB = 512
NQB = SEQ // QB  # 4 q-blocks per k-tile row


def build_nc():
    nc = bacc.Bacc("TRN2", target_bir_lowering=False, debug=False)

    qxT = nc.dram_tensor("qxT", [CQ, SEQ], BF16, kind="ExternalInput").ap()
    kvxT = nc.dram_tensor("kvxT", [CQ, SEQ], BF16, kind="ExternalInput").ap()
    biasT = nc.dram_tensor("biasT", [HPC, SEQ, SEQ], F32, kind="ExternalInput").ap()
    wq = nc.dram_tensor("wq", [CQ, HPC * DH], BF16, kind="ExternalInput").ap()
    wk = nc.dram_tensor("wk", [CQ, HPC * DH], BF16, kind="ExternalInput").ap()
    wv = nc.dram_tensor("wv", [CQ, HPC * DH], BF16, kind="ExternalInput").ap()
    wg = nc.dram_tensor("wg", [CQ, HPC * DH], BF16, kind="ExternalInput").ap()
    bg = nc.dram_tensor("bg", [HPC * DH, 1], F32, kind="ExternalInput").ap()
    wo = nc.dram_tensor("wo", [DH, HPC, 2, P], BF16, kind="ExternalInput").ap()
    # per-head unnormalized transposed partials + softmax denominators; the
    # division and cross-core summation happen on the host after the gather
    outs_d = [nc.dram_tensor(f"outT{h}", [CQ, SEQ], BF16, kind="ExternalOutput").ap()
              for h in range(HPC)]
    rs_d = nc.dram_tensor("rs", [1, HPC, SEQ], F32, kind="ExternalOutput").ap()

    EXP = mybir.ActivationFunctionType.Exp
    SIG = mybir.ActivationFunctionType.Sigmoid

    with tile.TileContext(nc) as tc:
        with ExitStack() as ctx:
            singles = ctx.enter_context(tc.tile_pool(name="singles", bufs=1))

            # ---- startup loads: K-path deps on sync, Q-path + the rest on
            # gpsimd, so both projection chains unblock in ~6us and neither
            # queue's bias stream starts before its loads are done. The
            # scalar queue stays empty (all queues share HBM bandwidth; its
            # engine pipeline must stay free for the exp stream).
            w_sbs = {}
            for name, src, eng in (("wk", wk, nc.sync), ("wq", wq, nc.scalar)):
                t = singles.tile([P, 2, P], BF16, tag=f"w_{name}")
                eng.dma_start(t, src.rearrange("(a p) c -> p a c", p=P))
                w_sbs[name] = t
            kvxT_sb = singles.tile([P, 2, SEQ], BF16)
            qxT_sb = singles.tile([P, 2, SEQ], BF16)
            for half in range(2):
                cs = bass.ts(half, SEQ // 2)
                nc.sync.dma_start(
                    kvxT_sb[:, :, cs],
                    kvxT.rearrange("(a p) c -> p a c", p=P)[:, :, cs])
                nc.scalar.dma_start(
                    qxT_sb[:, :, cs],
                    qxT.rearrange("(a p) c -> p a c", p=P)[:, :, cs])
            for name, src in (("wv", wv), ("wg", wg)):
                t = singles.tile([P, 2, P], BF16, tag=f"w_{name}")
                nc.scalar.dma_start(t, src.rearrange("(a p) c -> p a c", p=P))
                w_sbs[name] = t
            bg_sb = singles.tile([P, 1], F32)
            nc.scalar.dma_start(bg_sb, bg)
            wo_sb = singles.tile([DH, HPC, 2, P], BF16)
            nc.scalar.dma_start(wo_sb, wo)

            KT = singles.tile([P, SEQ], BF16)   # [2h x 64 d, k]
            QT = singles.tile([P, SEQ], BF16)   # [2h x 64 d, q]
            GT = singles.tile([P, SEQ], F32)    # gate, [2h x 64, q]
            V_sb = singles.tile([P, HPC, NKT, DH + 1], F32R)  # [k%128, h, kt, d|1]
            OG_sb = singles.tile([DH, HPC, SEQ], BF16)  # (o * g)^T per head
            rs_sb = singles.tile([1, HPC, SEQ], F32)    # softmax denominators
            os_sb = [singles.tile([P, 2, SEQ], BF16, tag=f"os{h}",
                                  name=f"os{h}")
                     for h in range(HPC)]  # outT staging [c%128, half, q]
            dummy = singles.tile([P, QB], BF16)
            nc.vector.memset(V_sb[:, :, :, DH:DH + 1].bitcast(F32), 1.0)
            nc.vector.memset(dummy, 1.0)

            biaspool = ctx.enter_context(tc.tile_pool(name="biasp", bufs=12))
            epool = ctx.enter_context(tc.tile_pool(name="ep", bufs=6))
            sbpool = ctx.enter_context(tc.tile_pool(name="sbp", bufs=4))
            spool = ctx.enter_context(
                tc.tile_pool(name="spsum", bufs=4, space="PSUM"))
            otpool = ctx.enter_context(
                tc.tile_pool(name="otpsum", bufs=2, space="PSUM"))

            def bias_dma(h, kt):
                t = biaspool.tile([P, SEQ], F32, tag="bias")
                if h == 0 and kt < 2:
                    # ring head start on the scalar queue, FIFO-ordered
                    # behind its startup loads
                    eng = nc.scalar
                else:
                    eng = nc.sync if kt % 2 == 0 else nc.gpsimd
                eng.dma_start(t, biasT[h, bass.ts(kt, P), :])
                return t

            # head 0's bias tiles stream right behind the startup loads;
            # head 1's are emitted inside head 0's loop (ring-depth lookahead)
            bias_t = [[None] * NKT for _ in range(HPC)]
            for kt in range(NKT):
                bias_t[0][kt] = bias_dma(0, kt)

            # ---- HAM warm-up: dummy matmuls while the input DMAs land, so
            # the PE clock is at 2.4 GHz when the projections start and
            # never sees a >1.7us gap again (the warm-state idle window).
            def emit_dummy(n):
                for _ in range(n):
                    ps = spool.tile([P, QB], F32, tag="s", name="warm")
                    nc.tensor.matmul(ps, dummy[:, 0:P], dummy,
                                     start=True, stop=True)

            emit_dummy(14)

            # ---- K/Q/G projections (V is emitted later as PE filler inside
            # head 0's attention loop). G runs here so ALL its sigmoids hit
            # the ACT engine before the first exp: a sigmoid interleaved
            # between exps forces a ~1.3us activation-table reload per switch.
            for wt, x_sb, dst in ((w_sbs["wk"], kvxT_sb, KT),
                                  (w_sbs["wq"], qxT_sb, QT)):
                for tt in range(SEQ // QB):
                    ps = spool.tile([P, QB], F32, tag="s")
                    nc.tensor.matmul(ps, wt[:, 0, :], x_sb[:, 0, bass.ts(tt, QB)],
                                     start=True, stop=False)
                    nc.tensor.matmul(ps, wt[:, 1, :], x_sb[:, 1, bass.ts(tt, QB)],
                                     start=False, stop=True)
                    nc.vector.tensor_copy(dst[:, bass.ts(tt, QB)], ps)

            def emit_vproj(g):
                # V out rows = tokens(k), cols = [kt-group x 2 heads x 64]
                ps = spool.tile([P, QB], F32, tag="s")
                for j in range(4):
                    kt = g * 4 + j
                    nc.tensor.matmul(ps[:, bass.ts(j, P)],
                                     kvxT_sb[:, 0, bass.ts(kt, P)], w_sbs["wv"][:, 0, :],
                                     start=True, stop=False)
                    nc.tensor.matmul(ps[:, bass.ts(j, P)],
                                     kvxT_sb[:, 1, bass.ts(kt, P)], w_sbs["wv"][:, 1, :],
                                     start=False, stop=True)
                nc.vector.tensor_copy(
                    V_sb[:, :, bass.ds(g * 4, 4), 0:DH],
                    ps.rearrange("p (a h d) -> p h a d", a=4, h=HPC))

            for tt in range(SEQ // QB):
                ps = spool.tile([P, QB], F32, tag="s", name="gproj_ps")
                nc.tensor.matmul(ps, w_sbs["wg"][:, 0, :],
                                 qxT_sb[:, 0, bass.ts(tt, QB)],
                                 start=True, stop=False)
                nc.tensor.matmul(ps, w_sbs["wg"][:, 1, :],
                                 qxT_sb[:, 1, bass.ts(tt, QB)],
                                 start=False, stop=True)
                nc.scalar.activation(GT[:, bass.ts(tt, QB)], ps, SIG, bias=bg_sb)

            def emit_av(h, kt, Es, OTs):
                for qb in range(NQB):
                    nc.tensor.matmul(
                        OTs[qb // 2][:, bass.ts(qb % 2, QB)],
                        V_sb[:, h, kt, :], Es[qb],
                        start=(kt == 0), stop=(kt == NKT - 1))

            def emit_outproj(h, half, qb, cast_eng):
                ps = spool.tile([P, QB], F32, tag="s", name="fin_ps")
                nc.tensor.matmul(ps, wo_sb[:, h, half, :],
                                 OG_sb[:, h, bass.ts(qb, QB)],
                                 start=True, stop=True)
                cast_eng_fn = (nc.scalar.copy if cast_eng == "act"
                               else lambda o, i: nc.vector.tensor_copy(o, i))
                cast_eng_fn(os_sb[h][:, half, bass.ts(qb, QB)], ps)

            hsl = [slice(h * DH, (h + 1) * DH) for h in range(HPC)]
            for h in range(HPC):
                OTs = [otpool.tile([DH + 1, 2 * QB], F32, name=f"OT{h}_{i}",
                                   tag="ot") for i in range(2)]
                prevE = None
                for kt in range(NKT):
                    # PE filler ahead of each S block: V projections during
                    # head 0, head 0's output projections during head 1
                    if h == 0:
                        if kt < 4:
                            emit_vproj(kt)
                    else:
                        if kt in (2, 4, 6, 8, 10, 12):
                            i = kt // 2 + 1
                            emit_outproj(0, i // 4, i % 4, "act")
                    Ss = []
                    for qb in range(NQB):
                        S = spool.tile([P, QB], F32, tag="s")
                        nc.tensor.matmul(S, KT[hsl[h], bass.ts(kt, P)],
                                         QT[hsl[h], bass.ts(qb, QB)],
                                         start=True, stop=True)
                        Ss.append(S)
                    if prevE is not None:
                        emit_av(h, kt - 1, prevE, OTs)
                    elif h == 1:
                        emit_outproj(0, 0, 0, "act")
                        emit_outproj(0, 0, 1, "act")
                    Es = []
                    for qb in range(NQB):
                        SB = sbpool.tile([P, QB], F32, tag="sb")
                        nc.vector.tensor_add(SB, Ss[qb],
                                             bias_t[h][kt][:, bass.ts(qb, QB)])
                        E = epool.tile([P, QB], F32R, tag="e")
                        nc.scalar.activation(E, SB, EXP)
                        Es.append(E)
                    prevE = Es
                    if h == 0:
                        bias_t[1][kt] = bias_dma(1, kt)
                emit_av(h, NKT - 1, prevE, OTs)
                # keepalive: pad the exp-chain/drain latency so the warm-state
                # HAM never sees a >1.7us PE gap at the head transition
                emit_dummy(5)
                # drain: exp-sum rows on ACT, gate muls on DVE; head 1's
                # output projections interleave with its drain halves
                for i in range(2):
                    qsl = bass.ts(i, 2 * QB)
                    nc.scalar.copy(rs_sb[:, h, qsl], OTs[i][DH:DH + 1, :])
                    nc.vector.tensor_mul(OG_sb[:, h, qsl], GT[hsl[h], qsl],
                                         OTs[i][0:DH, :])
                    if h == 1:
                        emit_dummy(3)
                        for half in range(2):
                            for qb in (2 * i, 2 * i + 1):
                                emit_outproj(1, half, qb,
                                             "act" if qb % 2 == 0 else "dve")

            # tail: the batched output stores
            nc.scalar.dma_start(
                outs_d[0].rearrange("(a p) c -> p a c", p=P), os_sb[0])
            nc.scalar.dma_start(outs_d[1][0:P, :], os_sb[1][:, 0, :])
            nc.sync.dma_start(outs_d[1][P:CQ, :], os_sb[1][:, 1, :])
            nc.sync.dma_start(rs_d, rs_sb)

    nc.compile()
    return nc


_NC = None
_NC_LOCK = threading.Lock()


def _get_nc():
    global _NC
    with _NC_LOCK:
        if _NC is None:
            _NC = build_nc()
        return _NC


def make_in_maps(q_x, kv_x, bias, w_q, w_k, w_v, w_g, b_g, w_o, b_o):
    del b_o  # added on the host after the gather
    q_x = np.asarray(q_x, dtype=np.float32)
    kv_x = np.asarray(kv_x, dtype=np.float32)
    bias = np.asarray(bias, dtype=np.float32)
    w_q = np.asarray(w_q, dtype=np.float32) * np.float32(0.125)  # fold 1/sqrt(64)
    w_k = np.asarray(w_k, dtype=np.float32)
    w_v = np.asarray(w_v, dtype=np.float32)
    w_g = np.asarray(w_g, dtype=np.float32)
    b_g = np.asarray(b_g, dtype=np.float32)
    w_o = np.asarray(w_o, dtype=np.float32)

    in_maps = []
    for c in range(N_CORES):
        b = c // (N_CORES // B)
        h0 = HPC * (c % (N_CORES // B))
        cols = slice(h0 * DH, (h0 + HPC) * DH)
        in_maps.append({
            "qxT": np.ascontiguousarray(q_x[b].T).astype(BF16_NP),
            "kvxT": np.ascontiguousarray(kv_x[b].T).astype(BF16_NP),
            "biasT": np.ascontiguousarray(bias[b, h0:h0 + HPC].swapaxes(1, 2)),
            "wq": np.ascontiguousarray(w_q[:, cols]).astype(BF16_NP),
            "wk": np.ascontiguousarray(w_k[:, cols]).astype(BF16_NP),
            "wv": np.ascontiguousarray(w_v[:, cols]).astype(BF16_NP),
            "wg": np.ascontiguousarray(w_g[:, cols]).astype(BF16_NP),
            "bg": np.ascontiguousarray(b_g[cols].reshape(HPC * DH, 1)),
            "wo": np.ascontiguousarray(
                w_o[cols, :].reshape(HPC, DH, 2, P).transpose(1, 0, 2, 3)
            ).astype(BF16_NP),
        })
    return in_maps


def gather_output(results, b_o):
    full = np.zeros((B, SEQ, CQ), dtype=np.float32)
    for c in range(N_CORES):
        b = c // (N_CORES // B)
        rs = results[c]["rs"][0]
        for h in range(HPC):
            outT = results[c][f"outT{h}"].astype(np.float32)  # [256, 2048]
            full[b] += (outT / rs[h][None, :]).T
    full += np.asarray(b_o, dtype=np.float32)
    return full


def kernel(**inputs):
    nc = _get_nc()
    in_maps = make_in_maps(**inputs)
    res = run_bass_kernel_spmd(nc, in_maps, core_ids=list(range(N_CORES)))
    return gather_output(res.results, inputs["b_o"])
